# revision 15
# baseline (speedup 1.0000x reference)
"""Trainium2 Bass kernel: mesh-guided deformation field (retrieval_knn).

Contract: kernel(**inputs) takes the FULL unsharded inputs from
reference.setup_inputs() and returns the FULL output, matching
reference.reference(**inputs) == (pts + pts_shifts, last_feat).

Strategy (8 NeuronCores, data-parallel over P):
  - each core handles P/8 = 4096 points for both batches (8192 points).
  - cdist+argmin: score s(p,v) = 2 p.m_v - |m_v|^2 computed on TensorE in
    fp16 hi/lo split arithmetic (exact to ~1e-7), K=16 rows, packed 4x via
    tile_position row groups.  argmin over v = Max8 + MaxIndex on VectorE
    over the score row (max s <=> min d2).  dist = |p|^2 - max s (exact).
  - gather of mesh_shift[idx] via gpsimd indirect DMA from a DRAM table,
    weighted by exp(-dist) on the point-major side, transposed to
    feature-major via TensorE transpose.
  - NeRF positional encodings: xf = S @ [pts; gshift] on TensorE (S folds
    1/2pi), fractional part via DVE mod, sin/cos via ScalarE Sin LUT on
    [-pi, pi) with sign folded into the host-permuted w0/w4 weights.
  - 8-layer MLP (skip concat at layer 4) in fp32r matmuls, feature-major
    activations [feat, points], ReLU+bias on ScalarE.
  - outputs written feature-major, host transposes back.
"""

import functools
import math
import sys

import numpy as np

sys.path.insert(0, "/opt/trn_rl_repo")

# ---------------------------------------------------------------- constants
B = 2
P = 32768
V = 5023
NCORES = 8
CP = P // NCORES          # points per core per batch = 4096
LAT = 32
HID = 128
NLAYERS = 8
DIM_IN = 110              # 63 + 15 + 32

GSZ = 512                 # point group (free dim of MLP tiles)
NGRP = CP // GSZ          # 8 groups per batch per core
VT = 512                  # vertex tile (matmul free dim)
NVT = (V + VT - 1) // VT  # 10
VPAD = NVT * VT           # 5120
VROW = V + 1              # payload table rows (5024)

F32 = np.float32
F16 = np.float16

_COMPILED = None


# ---------------------------------------------------------------- host prep
def _f16_split2(x64):
    """x ~= hi + lo with hi, lo fp16 (values exact in f64 arithmetic)."""
    hi = x64.astype(F16)
    lo = (x64 - hi.astype(np.float64)).astype(F16)
    return hi, lo


def _f16_split3(x64):
    a = x64.astype(F16)
    r = x64 - a.astype(np.float64)
    b = r.astype(F16)
    c = (r - b.astype(np.float64)).astype(F16)
    return a, b, c


def _perm_sign():
    """ref initial column -> (my row, sign). my layout (engine partition-base
    constraint: ACT writes must start at 32-aligned partitions):
    0-35 cos(xf), 36-38 pts, 39-41 gshift, 42-63 latent[0:22],
    64-99 sin(xf), 100-109 latent[22:32].
    xf index m: pts (l,c) -> 3l+c (l<10); gshift (l,c) -> 30+3l+c (l<2).
    r = t - rne(t) in [-0.5, 0.5], so Sin(2pi*r) = sin(2pi*t) directly.
    """
    rows = np.zeros(DIM_IN, dtype=np.int64)
    sign = np.ones(DIM_IN, dtype=np.float64)
    for c in range(3):
        rows[c] = 36 + c                 # pts
        rows[63 + c] = 39 + c            # gshift
    for i in range(LAT):
        rows[78 + i] = (42 + i) if i < 22 else (78 + i)
    for l in range(10):
        for j in range(6):
            ref = 3 + 6 * l + j
            m = 3 * l + (j % 3)
            rows[ref] = (64 + m) if j < 3 else m
    for l in range(2):
        for j in range(6):
            ref = 66 + 6 * l + j
            m = 30 + 3 * l + (j % 3)
            rows[ref] = (64 + m) if j < 3 else m
    return rows, sign


def _prep(inputs):
    """Build all per-core DRAM input arrays. Returns list of dicts."""
    pts = np.asarray(inputs["pts"], dtype=F32)                   # [B,P,3]
    mesh = np.asarray(inputs["deformed_mesh_pts"], dtype=F32)    # [B,V,3]
    cano = np.asarray(inputs["cano_verts"], dtype=F32)           # [1,V,3]
    lat = np.asarray(inputs["motion_latent"], dtype=F32)         # [B,P,LAT]

    p64 = pts.astype(np.float64)
    m64 = mesh.astype(np.float64)

    # ---- cdist lhs rows [B, 16, P] fp16
    phi, plo = _f16_split2(p64)                                  # [B,P,3]
    lhs = np.zeros((B, 16, P), dtype=F16)
    lhs[:, 0:3, :] = 1.0
    lhs[:, 3:6, :] = phi.transpose(0, 2, 1)
    lhs[:, 6:9, :] = phi.transpose(0, 2, 1)
    lhs[:, 9:12, :] = plo.transpose(0, 2, 1)
    lhs[:, 12:15, :] = plo.transpose(0, 2, 1)

    # ---- cdist rhs rows [B, NVT, 128, VT] fp16 (replicated at 4 offsets)
    m2 = (m64 ** 2).sum(-1)                                      # [B,V]
    r0, r1, r2 = _f16_split3(-m2)
    mhi, mlo = _f16_split2(m64)
    rhs_rows = np.zeros((B, 16, VPAD), dtype=F16)
    rhs_rows[:, 0, :V] = r0
    rhs_rows[:, 1, :V] = r1
    rhs_rows[:, 2, :V] = r2
    rhs_rows[:, 3:6, :V] = 2.0 * mhi.transpose(0, 2, 1)
    rhs_rows[:, 6:9, :V] = 2.0 * mlo.transpose(0, 2, 1)
    rhs_rows[:, 9:12, :V] = 2.0 * mhi.transpose(0, 2, 1)
    rhs_rows[:, 12:15, :V] = 2.0 * mlo.transpose(0, 2, 1)
    rhs_rows[:, 0, V:] = -60000.0                                # pad loses
    verts = np.zeros((B, NVT, 128, VT), dtype=F16)
    vtiles = rhs_rows.reshape(B, 16, NVT, VT)
    for off in (0, 32, 64, 96):
        verts[:, :, off:off + 16, :] = vtiles.transpose(0, 2, 1, 3)

    # ---- feature-major points / |p|^2 / latent
    pts_fm = np.ones((B, 4, P), dtype=F32)                       # [B,4,P]
    pts_fm[:, 0:3, :] = pts.transpose(0, 2, 1)
    psq = (p64 ** 2).sum(-1).astype(F32)                         # [B,P]
    # [B, 128, P/128]: [b, i, j] = |p|^2 of point j*128+i
    psq_t = psq.reshape(B, P // 128, 128).transpose(0, 2, 1).copy()
    lat_fm = lat.transpose(0, 2, 1).copy()                       # [B,LAT,P]

    # ---- gather payload [B, VROW, 4]
    shift = (cano.astype(np.float64) - m64).astype(F32)          # [B,V,3]
    pay = np.zeros((B, VROW, 4), dtype=F32)
    pay[:, :V, 0:3] = shift

    # ---- weights
    rows, sign = _perm_sign()
    w0 = np.asarray(inputs["w0"], dtype=F32)                     # [128,110]
    w4 = np.asarray(inputs["w4"], dtype=F32)                     # [128,238]
    w0t = np.zeros((112, HID), dtype=F32)
    w4at = np.zeros((112, HID), dtype=F32)
    for ref in range(DIM_IN):
        w0t[rows[ref], :] = w0[:, ref] * sign[ref]
        w4at[rows[ref], :] = w4[:, ref] * sign[ref]
    w4bt = np.ascontiguousarray(w4[:, DIM_IN:].T)                # [128,128]
    wmid = {}
    for i in (1, 2, 3, 5, 6, 7):
        wmid[i] = np.ascontiguousarray(
            np.asarray(inputs[f"w{i}"], dtype=F32).T)            # [128,128]
    wout = np.asarray(inputs["w_out"], dtype=F32)                # [3,128]
    woutt = np.zeros((HID, 4), dtype=F32)
    woutt[:, 0:3] = wout.T
    ball = np.zeros((HID, 9), dtype=F32)
    for i in range(NLAYERS):
        ball[:, i] = np.asarray(inputs[f"b{i}"], dtype=F32)
    ball[0:3, 8] = np.asarray(inputs["b_out"], dtype=F32)

    # ---- S matrix [7, 72]: rows match pg rows [x,y,z,1,gx,gy,gz];
    # cols 0-35 sin-args, 36-71 cos-args (= sin-args + 0.25 via ones row).
    # xf in turns: coeff 2^l / 2pi.
    smat = np.zeros((7, 72), dtype=F32)
    for l in range(10):
        for c in range(3):
            smat[c, 3 * l + c] = (2.0 ** l) / (2.0 * math.pi)
            smat[c, 36 + 3 * l + c] = (2.0 ** l) / (2.0 * math.pi)
    for l in range(2):
        for c in range(3):
            smat[4 + c, 30 + 3 * l + c] = (2.0 ** l) / (2.0 * math.pi)
            smat[4 + c, 66 + 3 * l + c] = (2.0 ** l) / (2.0 * math.pi)
    smat[3, 36:72] = 0.25

    shared = {
        "w0t": w0t, "w4at": w4at, "w4bt": w4bt, "woutt": woutt,
        "ball": ball, "smat": smat,
        "w1t": wmid[1], "w2t": wmid[2], "w3t": wmid[3],
        "w5t": wmid[5], "w6t": wmid[6], "w7t": wmid[7],
        "verts": verts, "pay0": pay[0], "pay1": pay[1],
    }
    in_maps = []
    for c in range(NCORES):
        sl = slice(c * CP, (c + 1) * CP)
        blk = slice(c * (CP // 128), (c + 1) * (CP // 128))
        m = dict(shared)
        m["lhs"] = np.ascontiguousarray(lhs[:, :, sl])           # [B,16,CP]
        m["pts_fm"] = np.ascontiguousarray(pts_fm[:, :, sl])     # [B,3,CP]
        m["psq_t"] = np.ascontiguousarray(psq_t[:, :, blk])      # [B,128,CP/128]
        m["lat_fm"] = np.ascontiguousarray(lat_fm[:, :, sl])     # [B,LAT,CP]
        in_maps.append(m)
    return in_maps


# ---------------------------------------------------------------- bass build
def _build(repeat=1):
    import concourse.bass as bass
    import concourse.mybir as mybir
    import concourse.tile as tile
    from concourse import bacc
    from concourse.masks import make_identity

    dt = mybir.dt
    AF = mybir.ActivationFunctionType
    ALU = mybir.AluOpType
    PI = math.pi

    import os
    ngrp = int(os.environ.get("KERNEL_NGRP", NGRP))
    nbat = int(os.environ.get("KERNEL_NB", B))

    nc = bacc.Bacc("TRN2", target_bir_lowering=False, debug=False,
                   enable_asserts=False, num_devices=NCORES)

    def din(name, shape, dtype=dt.float32):
        return nc.dram_tensor(name, shape, dtype, kind="ExternalInput").ap()

    def dout(name, shape, dtype=dt.float32):
        return nc.dram_tensor(name, shape, dtype, kind="ExternalOutput").ap()

    lhs_d = din("lhs", [B, 16, CP], dt.float16)
    verts_d = din("verts", [B, NVT, 128, VT], dt.float16)
    pts_d = din("pts_fm", [B, 4, CP])
    psq_d = din("psq_t", [B, 128, CP // 128])
    lat_d = din("lat_fm", [B, LAT, CP])
    pay_d = [din("pay0", [VROW, 4]), din("pay1", [VROW, 4])]
    w0_d = din("w0t", [112, HID])
    w4a_d = din("w4at", [112, HID])
    w4b_d = din("w4bt", [HID, HID])
    wout_d = din("woutt", [HID, 4])
    ball_d = din("ball", [HID, 9])
    smat_d = din("smat", [7, 72])
    wmid_d = {i: din(f"w{i}t", [HID, HID]) for i in (1, 2, 3, 5, 6, 7)}

    out1_d = dout("out1", [B, 3, CP])
    feat_d = dout("feat", [B, HID, CP])


    with tile.TileContext(nc) as tc:
        with (
            tc.tile_pool(name="const", bufs=1) as cp,
            tc.tile_pool(name="score", bufs=1) as scp,
            tc.tile_pool(name="work", bufs=2) as wp,
            tc.tile_pool(name="acts", bufs=3) as xp,
            tc.tile_pool(name="pscore", bufs=4, space="PSUM") as pscore,
            tc.tile_pool(name="ptrans", bufs=1, space="PSUM") as ptrans,
            tc.tile_pool(name="pxf", bufs=1, space="PSUM") as pxf,
            tc.tile_pool(name="pmm", bufs=2, space="PSUM") as pmm,
        ):
            # ---------------- constants / weights
            ident = cp.tile([128, 128], dt.float32)
            make_identity(nc, ident[:])
            zbias = cp.tile([128, 1], dt.float32, tag="zbias")
            nc.gpsimd.memset(zbias[:], 0.0)
            verts_sb = []
            for b in range(B):
                vt = cp.tile([128, NVT * VT], dt.float16, tag=f"verts{b}")
                for t in range(NVT):
                    nc.sync.dma_start(out=vt[:, t * VT:(t + 1) * VT],
                                      in_=verts_d[b, t])
                verts_sb.append(vt)
            w0_sb = cp.tile([112, HID], dt.float32, tag="w0")
            nc.sync.dma_start(out=w0_sb[:], in_=w0_d[:])
            w4a_sb = cp.tile([112, HID], dt.float32, tag="w4a")
            nc.sync.dma_start(out=w4a_sb[:], in_=w4a_d[:])
            w4b_sb = cp.tile([HID, HID], dt.float32, tag="w4b")
            nc.sync.dma_start(out=w4b_sb[:], in_=w4b_d[:])
            wout_sb = cp.tile([HID, 4], dt.float32, tag="wout")
            nc.sync.dma_start(out=wout_sb[:], in_=wout_d[:])
            ball_sb = cp.tile([HID, 9], dt.float32, tag="ball")
            nc.sync.dma_start(out=ball_sb[:], in_=ball_d[:])
            smat_sb = cp.tile([7, 72], dt.float32, tag="smat")
            nc.sync.dma_start(out=smat_sb[:], in_=smat_d[:])
            wmid_sb = {}
            for i in (1, 2, 3, 5, 6, 7):
                w = cp.tile([HID, HID], dt.float32, tag=f"w{i}")
                nc.sync.dma_start(out=w[:], in_=wmid_d[i][:])
                wmid_sb[i] = w

            for _rep in range(repeat):
              for b in range(nbat):
                for g in range(ngrp):
                    gsl = slice(g * GSZ, (g + 1) * GSZ)
                    # -------- cdist lhs [128, 128] f16
                    lhs_sb = wp.tile([128, 128], dt.float16, tag="lhs")
                    for k in range(4):
                        nc.sync.dma_start(
                            out=lhs_sb[32 * k:32 * k + 16, :],
                            in_=lhs_d[b, :, g * GSZ + 128 * k:
                                      g * GSZ + 128 * (k + 1)])
                    # -------- scores -> SBUF row per block
                    score = scp.tile([128, 4 * VPAD], dt.float32, tag="score")
                    for t in range(NVT):
                        for k in range(4):
                            ps = pscore.tile([128, VT], dt.float32,
                                             tag="psc")
                            nc.tensor.matmul(
                                ps[:], lhsT=lhs_sb[32 * k:32 * k + 16, :],
                                rhs=verts_sb[b][32 * k:32 * k + 16,
                                                t * VT:(t + 1) * VT],
                                start=True, stop=True,
                                tile_position=(32 * k, 0))
                            nc.scalar.copy(
                                out=score[:, k * VPAD + t * VT:
                                          k * VPAD + (t + 1) * VT],
                                in_=ps[:])
                    # -------- argmin per block
                    mxs = wp.tile([128, 32], dt.float32, tag="mxs")
                    mis = wp.tile([128, 32], dt.uint32, tag="mis")
                    for k in range(4):
                        nc.vector.max(
                            mxs[:, 8 * k:8 * k + 8],
                            score[:, k * VPAD:(k + 1) * VPAD])
                        nc.vector.max_index(
                            mis[:, 8 * k:8 * k + 8],
                            mxs[:, 8 * k:8 * k + 8],
                            score[:, k * VPAD:(k + 1) * VPAD])
                    # -------- dist / weight  [128, 4]
                    psq_sb = wp.tile([128, 4], dt.float32, tag="psq")
                    nc.sync.dma_start(out=psq_sb[:],
                                      in_=psq_d[b, :, 4 * g:4 * g + 4])
                    dist = wp.tile([128, 4], dt.float32, tag="dist")
                    nc.vector.tensor_tensor(
                        out=dist[:], in0=psq_sb[:], in1=mxs[:, 0:32:8],
                        op=ALU.subtract)
                    wgt = wp.tile([128, 4], dt.float32, tag="wgt")
                    nc.vector.tensor_scalar_max(wgt[:], dist[:], 0.0)
                    nc.scalar.activation(wgt[:], wgt[:], AF.Exp, scale=-1.0)
                    # -------- gather + weight + transpose
                    gsh = wp.tile([4, GSZ], dt.float32, tag="gsh")
                    for k in range(4):
                        gat = wp.tile([128, 4], dt.float32, tag="gat")
                        nc.gpsimd.indirect_dma_start(
                            out=gat[:], out_offset=None,
                            in_=pay_d[b][:],
                            in_offset=bass.IndirectOffsetOnAxis(
                                ap=mis[:, 8 * k:8 * k + 1], axis=0))
                        nc.vector.tensor_scalar_mul(
                            gat[:], gat[:], wgt[:, k:k + 1])
                        pst = ptrans.tile([4, 128], dt.float32, tag="pst")
                        nc.tensor.transpose(pst[:], gat[:], ident[:])
                        nc.vector.tensor_copy(
                            out=gsh[:, 128 * k:128 * (k + 1)], in_=pst[:])
    # -------- initial features [110, GSZ]
                    init = xp.tile([128, GSZ], dt.float32, tag="init")
                    pg = wp.tile([8, GSZ], dt.float32, tag="pg")
                    nc.sync.dma_start(out=pg[0:4, :], in_=pts_d[b, :, gsl])
                    nc.sync.dma_start(out=pg[4:7, :], in_=gsh[0:3, :])
                    nc.sync.dma_start(out=init[36:39, :],
                                      in_=pts_d[b, 0:3, gsl])
                    nc.sync.dma_start(out=init[39:42, :], in_=gsh[0:3, :])
                    nc.sync.dma_start(out=init[42:64, :],
                                      in_=lat_d[b, 0:22, gsl])
                    nc.sync.dma_start(out=init[100:110, :],
                                      in_=lat_d[b, 22:LAT, gsl])
                    MAGIC = 12582912.0  # 1.5 * 2**23: t+M-M = rne(t)
                    for half, dst in ((0, 64), (36, 0)):
                        ps_xf = pxf.tile([36, GSZ], dt.float32, tag="pxf")
                        nc.tensor.matmul(
                            ps_xf[:], lhsT=smat_sb[:, half:half + 36],
                            rhs=pg[0:7, :], start=True, stop=True)
                        kk = wp.tile([36, GSZ], dt.float32, tag="kk")
                        nc.vector.tensor_scalar(
                            out=kk[:], in0=ps_xf[:], scalar1=MAGIC,
                            scalar2=-MAGIC, op0=ALU.add, op1=ALU.add)
                        rr = wp.tile([36, GSZ], dt.float32, tag="rr")
                        nc.vector.tensor_tensor(
                            out=rr[:], in0=ps_xf[:], in1=kk[:],
                            op=ALU.subtract)
                        nc.scalar.activation(
                            init[dst:dst + 36, :], rr[:], AF.Sin,
                            bias=zbias[0:36, :], scale=2.0 * PI)
                    # -------- MLP
                    x = None
                    for i in range(NLAYERS):
                        ps = pmm.tile([128, GSZ], dt.float32, tag="pm")
                        if i == 0:
                            nc.tensor.matmul(
                                ps[:], lhsT=w0_sb[0:110, :],
                                rhs=init[0:110, :],
                                start=True, stop=True)
                        elif i == 4:
                            nc.tensor.matmul(
                                ps[:], lhsT=w4a_sb[0:110, :],
                                rhs=init[0:110, :],
                                start=True, stop=False)
                            nc.tensor.matmul(
                                ps[:], lhsT=w4b_sb[:],
                                rhs=x[:],
                                start=False, stop=True)
                        else:
                            nc.tensor.matmul(
                                ps[:], lhsT=wmid_sb[i][:],
                                rhs=x[:],
                                start=True, stop=True)
                        if i == NLAYERS - 1:
                            feat = wp.tile([128, GSZ], dt.float32, tag="feat")
                            nc.vector.tensor_scalar_add(
                                feat[:], ps[:], ball_sb[:, 7:8])
                            nc.sync.dma_start(out=feat_d[b, :, gsl],
                                              in_=feat[:])
                        xn = xp.tile([128, GSZ], dt.float32, tag="x")
                        nc.scalar.activation(xn[:], ps[:], AF.Relu,
                                             bias=ball_sb[:, i:i + 1],
                                             scale=1.0)
                        x = xn
                    ps_o = pmm.tile([128, GSZ], dt.float32, tag="pm")
                    nc.tensor.matmul(ps_o[0:4, :], lhsT=wout_sb[:],
                                     rhs=x[:],
                                     start=True, stop=True)
                    o1 = wp.tile([4, GSZ], dt.float32, tag="o1")
                    nc.vector.scalar_tensor_tensor(
                        out=o1[0:3, :], in0=ps_o[0:3, :],
                        scalar=ball_sb[0:3, 8:9], op0=ALU.add,
                        in1=gsh[0:3, :], op1=ALU.add)
                    nc.vector.tensor_tensor(
                        out=o1[0:3, :], in0=o1[0:3, :], in1=pg[0:3, :],
                        op=ALU.add)
                    nc.sync.dma_start(out=out1_d[b, :, gsl], in_=o1[0:3, :])

    nc.compile()
    return nc


def _get_compiled():
    global _COMPILED
    if _COMPILED is None:
        _COMPILED = _build()
    return _COMPILED


# ---------------------------------------------------------------- entry
def kernel(**inputs):
    from concourse.bass_utils import run_bass_kernel_spmd

    nc = _get_compiled()
    in_maps = _prep(inputs)
    res = run_bass_kernel_spmd(nc, in_maps, list(range(NCORES)))
    outs = res.results

    out1 = np.zeros((B, P, 3), dtype=F32)
    feat = np.zeros((B, P, HID), dtype=F32)
    for c in range(NCORES):
        sl = slice(c * CP, (c + 1) * CP)
        out1[:, sl, :] = outs[c]["out1"].transpose(0, 2, 1)
        feat[:, sl, :] = outs[c]["feat"].transpose(0, 2, 1)
    return out1, feat
